# revision 34
# baseline (speedup 1.0000x reference)
"""Cross-attention kernel for Trainium2, sharded over 8 NeuronCores.

Problem (per reference):
  q = wq @ x_q + bq ; k = wk @ x_kv + bk ; v = wv @ x_kv + bv   (1x1 convs)
  per head: attn = softmax(q^T k / sqrt(hd)) ; out = attn @ v^T
  y = wo @ out + bo

Sharding: core c -> (batch b = c // 4, head n = c % 4). Each core runs one
head's full attention and produces the partial output projection
y_part = wo[:, head] @ out_head; the host sums the 4 head partials per batch.

Device-side simplifications (all mathematically exact):
  * bk drops out entirely: a per-query constant shift of the logits cancels
    in softmax.
  * bv folds into the output bias: sum_j softmax_ij = 1, so v-bias
    contributes wo_col @ bv, added to bo on the host.
  * scale 1/8 folds into wq/bq on the host.
  * no max-subtraction: logits are ~N(0,1) (max |logit| < ~6), exp is safe
    in fp32.
  * softmax denominator comes from a ones-column appended to v^T in the AV
    matmul; normalization happens after AV on [64, S] instead of [S, S].

Layouts: logits are computed transposed, S^T[j, i] (k stationary, q moving),
so the exp'd tile feeds the AV matmul directly with j on partitions — no
transposes anywhere. v^T is produced directly by using x_kv chunks as the
stationary operand of the v projection.
"""

import numpy as np
import ml_dtypes

import concourse.bacc as bacc
import concourse.mybir as mybir
import concourse.tile as tile
from concourse.bass_utils import run_bass_kernel_spmd

F32 = mybir.dt.float32
BF16 = mybir.dt.bfloat16

B, C, HGT, WID = 2, 256, 64, 64
S = HGT * WID  # 4096 pixels
NH, HD = 4, 64
NCORES = 8
P = 128
IC = 1024  # i-chunk width (2 PSUM banks)
NI = S // IC  # 4
NJ = S // P  # 32 j-blocks
SCALE = HD ** -0.5
EXP_W = 1024  # free width of one exp instruction (must divide IC)


def _emit(tc):
    nc = tc.nc
    xq = nc.dram_tensor("xq", [2, P, S], BF16, kind="ExternalInput").ap()
    xkv = nc.dram_tensor("xkv", [2, P, S], BF16, kind="ExternalInput").ap()
    wqT = nc.dram_tensor("wqT", [2, P, HD], BF16, kind="ExternalInput").ap()
    wkvT = nc.dram_tensor("wkvT", [2, P, P], BF16, kind="ExternalInput").ap()
    woT = nc.dram_tensor("woT", [HD, C], BF16, kind="ExternalInput").ap()
    bq = nc.dram_tensor("bq", [HD, 1], F32, kind="ExternalInput").ap()
    y = nc.dram_tensor("y", [2, P, S], F32, kind="ExternalOutput").ap()
    yden = nc.dram_tensor("yden", [1, S], F32, kind="ExternalOutput").ap()

    with (
        tc.tile_pool(name="const", bufs=1) as cpool,
        tc.tile_pool(name="xp", bufs=1) as xpool,
        tc.tile_pool(name="qkv", bufs=1) as qpool,
        tc.tile_pool(name="es", bufs=3) as epool,
        tc.tile_pool(name="epi", bufs=2) as fpool,
        tc.tile_pool(name="ps", bufs=2, space="PSUM") as pp,
    ):
        # ---- weights / constants into SBUF ----
        wq_sb = cpool.tile([P, 2 * HD], BF16)
        wkv_sb = cpool.tile([P, 2 * P], BF16)
        # weights go via the gpsimd SWDGE queue — a third DMA path that
        # doesn't block the two HWDGE rings carrying the 4MB of activations
        for cch in range(2):
            nc.gpsimd.dma_start(wq_sb[:, cch * HD:(cch + 1) * HD], wqT[cch])
            nc.gpsimd.dma_start(wkv_sb[:, cch * P:(cch + 1) * P], wkvT[cch])
        wo_sb = cpool.tile([HD, C], BF16)
        nc.gpsimd.dma_start(wo_sb[:], woT)
        bq_sb = cpool.tile([HD, 1], F32)
        nc.gpsimd.dma_start(bq_sb[:], bq)
        # Zero bias for exp via memset: a float bias would become a DMA'd
        # const tensor whose transfer queues behind the 4MB input DMAs,
        # delaying the first exp (and idling the PE into a HAM down-clock).
        zbias_sb = cpool.tile([P, 1], F32)
        nc.vector.memset(zbias_sb[:], 0.0)
        # PE warmup burst: ~3.5us of dense matmuls on scratch data while the
        # input DMAs are still in flight. The activity monitor promotes the
        # PE to 2.4GHz after ~3.4us of sustained streaming; without this the
        # whole projection phase runs at the cold 1.2GHz clock.
        wrm_sb = cpool.tile([P, 512], BF16)
        nc.vector.memset(wrm_sb[:], 0.0)
        for w in range(3):
            wp = pp.tile([P, 512], F32, tag="s", bufs=2, name="wp")
            nc.tensor.matmul(wp[:], wrm_sb[:, 0:P], wrm_sb[:],
                             start=True, stop=True)
        # Warmup exp so the ~2.7us activation-table load happens during the
        # projection phase, not in front of the first real exp.
        warm_sb = cpool.tile([P, 1], BF16)
        nc.scalar.activation(warm_sb[:], zbias_sb[:],
                             mybir.ActivationFunctionType.Exp,
                             bias=zbias_sb[:])

        # ---- activations into SBUF ----
        xq_sb = [xpool.tile([P, S], BF16, tag=f"xq{i}", name=f"xq_sb{i}")
                 for i in range(2)]
        xkv_sb = [xpool.tile([P, S], BF16, tag=f"xkv{i}", name=f"xkv_sb{i}")
                  for i in range(2)]
        # x_kv lands first (k and v^T projections run first); quarter-DMAs
        # let the projections start before the whole tensor arrives.
        QW = S // 2
        # Chunk 0 on the sync HWDGE ring, chunk 1 on the scalar-engine ring:
        # both rings stream in parallel, halving time-to-arrival. Half-sized
        # transfers: arriving a bit later but with fewer per-DMA overheads —
        # the projections then stream without arrival gaps (a >3.4us PE gap
        # would demote the clock).
        for qt in range(2):
            qsl = slice(qt * QW, (qt + 1) * QW)
            nc.sync.dma_start(xkv_sb[0][:, qsl], xkv[0][:, qsl])
            nc.scalar.dma_start(xkv_sb[1][:, qsl], xkv[1][:, qsl])
        for qt in range(2):
            qsl = slice(qt * QW, (qt + 1) * QW)
            nc.sync.dma_start(xq_sb[0][:, qsl], xq[0][:, qsl])
            nc.scalar.dma_start(xq_sb[1][:, qsl], xq[1][:, qsl])

        # q/k are zero-padded to 128 partitions and v^T blocks to 128
        # columns so every LDWEIGHTS is a full [128,128] bf16 load (fast
        # weight-load eligible); the zero rows contribute nothing.
        q_sb = qpool.tile([P, S], BF16)
        k_sb = qpool.tile([P, S], BF16)
        # v^T blocks: [j-part, (block, 128)]; col 64 = ones (denominator),
        # cols 65:128 zero.
        va_sb = qpool.tile([P, NJ * P], BF16)
        nc.vector.memset(va_sb[:], 0.0)
        va_v = va_sb.rearrange("p (j c) -> p j c", c=P)
        nc.vector.memset(va_v[:, :, HD:HD + 1], 1.0)
        # pad memsets run here: the DVE is idle while the input DMAs stream
        nc.vector.memset(q_sb[HD:P, :], 0.0)
        nc.vector.memset(k_sb[HD:P, :], 0.0)

        # ---- fused k+v projection: stationary [wk^T | wv^T] gives k in
        # PSUM rows 0:64 and v in rows 64:128 from the same matmul pass
        # (dense, high PE duty: together with the warmup burst this promotes
        # the PE clock early). v is then moved into the [j-part, block, hd]
        # layout the AV matmul needs by one hardware DMA transpose — a
        # direct x_kv-stationary v^T projection would be 64 tiny matmuls
        # dominated by LDWEIGHTS (~15us of low-duty PE time).
        v_sb = qpool.tile([P, S], BF16)
        for t in range(S // 512):
            sl = slice(t * 512, (t + 1) * 512)
            kvp = pp.tile([P, 512], F32, tag="s", bufs=2)
            nc.tensor.matmul(kvp[:], wkv_sb[:, 0:P], xkv_sb[0][:, sl],
                             start=True, stop=False)
            nc.tensor.matmul(kvp[:], wkv_sb[:, P:2 * P], xkv_sb[1][:, sl],
                             start=False, stop=True)
            nc.vector.tensor_copy(k_sb[0:HD, sl], kvp[0:HD, :])
            # v copy runs on the (otherwise idle) scalar engine
            nc.scalar.activation(v_sb[HD:P, sl], kvp[HD:P, :],
                                 mybir.ActivationFunctionType.Copy)
            if t % 2 == 1:
                # piece-wise transpose as soon as each 1024-col span of v is
                # ready (one big transpose at the end would finish after the
                # first AV matmul needs it, stalling the loop start)
                tp = t // 2
                nc.scalar.dma_start_transpose(
                    out=va_v[:, tp * 8:(tp + 1) * 8, 0:HD],
                    in_=v_sb[HD:P, tp * 1024:(tp + 1) * 1024])

        # ---- q projection (scale and bias folded in on the host) ----
        for t in range(S // 512):
            sl = slice(t * 512, (t + 1) * 512)
            qp = pp.tile([HD, 512], F32, tag="s", bufs=2)
            nc.tensor.matmul(qp[:], wq_sb[:, 0:HD], xq_sb[0][:, sl],
                             start=True, stop=False)
            nc.tensor.matmul(qp[:], wq_sb[:, HD:2 * HD], xq_sb[1][:, sl],
                             start=False, stop=True)
            nc.vector.tensor_scalar_add(q_sb[0:HD, sl], qp[:], bq_sb[:])



        # ---- attention, i-chunk at a time ----
        # Softmax normalization is deferred to the host: division by the
        # denominator commutes with the output projection, so the device
        # ships y_un = wo_col @ (exp(S^T)^T V)^T plus the per-pixel
        # denominators, and the host computes y_un / den + bias. This keeps
        # any long dependency chain (reciprocal etc.) out of the in-order PE
        # stream — a multi-us PE stall makes the HW activity monitor halve
        # the PE clock for the rest of the kernel.
        pend = [None] * NI  # per chunk: unnormalized out^T awaiting out-proj

        def epilogue_part2(i):
            # out-projection of the (unnormalized) attention output
            outt = pend[i]
            for oh in range(2):
                for h in range(IC // 512):
                    yp = pp.tile([P, 512], F32, tag="av", bufs=2, name="yp")
                    nc.tensor.matmul(yp[:], wo_sb[:, oh * P:(oh + 1) * P],
                                     outt[:, h * 512:(h + 1) * 512],
                                     start=True, stop=True)
                    ys = fpool.tile([P, 512], F32, name="ys")
                    nc.vector.tensor_copy(ys[:], yp[:])
                    eng = nc.sync if oh == 0 else nc.scalar
                    eng.dma_start(
                        y[oh][:, i * IC + h * 512:i * IC + (h + 1) * 512],
                        ys[:])

        for i in range(NI):
            av = pp.tile([P, IC], F32, tag="av", bufs=2)
            for j in range(NJ):
                if i > 0 and j == 8:
                    epilogue_part2(i - 1)
                jb = slice(j * P, (j + 1) * P)
                st = pp.tile([P, IC], F32, tag="s", bufs=2)
                for h in range(IC // 512):
                    isl = slice(i * IC + h * 512, i * IC + (h + 1) * 512)
                    nc.tensor.matmul(st[:, h * 512:(h + 1) * 512],
                                     k_sb[:, jb], q_sb[:, isl],
                                     start=True, stop=True)
                et = epool.tile([P, IC], BF16)
                for h in range(IC // EXP_W):
                    esl = slice(h * EXP_W, (h + 1) * EXP_W)
                    nc.scalar.activation(et[:, esl], st[:, esl],
                                         mybir.ActivationFunctionType.Exp,
                                         bias=zbias_sb[:])
                vbase = j * P
                for h in range(IC // 512):
                    nc.tensor.matmul(av[:, h * 512:(h + 1) * 512],
                                     va_sb[:, vbase:vbase + P],
                                     et[:, h * 512:(h + 1) * 512],
                                     start=(j == 0), stop=(j == NJ - 1))

            # epilogue part 1: DVE only — drain PSUM and free the av slot
            outt = fpool.tile([HD, IC], BF16)
            nc.vector.tensor_copy(outt[:], av[0:HD, :])
            den = fpool.tile([HD + 1, IC], F32, name="den")
            nc.vector.tensor_copy(den[HD:HD + 1, :], av[HD:HD + 1, :])
            nc.gpsimd.dma_start(yden[:, i * IC:(i + 1) * IC],
                                den[HD:HD + 1, :])
            pend[i] = outt

        epilogue_part2(NI - 1)


def build():
    nc = bacc.Bacc("TRN2", target_bir_lowering=False, debug=False,
                   enable_asserts=False)
    with tile.TileContext(nc) as tc:
        _emit(tc)
    nc.compile()
    return nc


_NC_CACHE = []


def _get_nc():
    if not _NC_CACHE:
        _NC_CACHE.append(build())
    return _NC_CACHE[0]


def make_in_maps(x_q, x_kv, wq, bq, wk, bk, wv, bv, wo, bo):
    bf = ml_dtypes.bfloat16
    in_maps = []
    bo_effs = []
    for c in range(NCORES):
        b, n = divmod(c, NH)
        hs = slice(n * HD, (n + 1) * HD)
        wq_h = wq[hs].astype(np.float64) * SCALE
        bo_eff = wo[:, hs].astype(np.float64) @ bv[hs].astype(np.float64)
        if n == 0:
            bo_eff = bo_eff + bo.astype(np.float64)
        bo_effs.append(bo_eff.astype(np.float32))
        in_maps.append({
            "xq": np.ascontiguousarray(
                x_q[b].reshape(C, S).reshape(2, P, S)).astype(bf),
            "xkv": np.ascontiguousarray(
                x_kv[b].reshape(C, S).reshape(2, P, S)).astype(bf),
            "wqT": np.ascontiguousarray(wq_h.T.reshape(2, P, HD)).astype(bf),
            "wkvT": np.ascontiguousarray(
                np.concatenate([wk[hs].T, wv[hs].T], axis=1)
                .reshape(2, P, P)).astype(bf),
            "woT": np.ascontiguousarray(wo[:, hs].T).astype(bf),
            "bq": (bq[hs].astype(np.float64) * SCALE
                   ).astype(np.float32).reshape(HD, 1),
        })
    return in_maps, bo_effs


def assemble_output(results, bo_effs):
    # y_core is the unnormalized head partial; divide by the softmax
    # denominator and add the (host-folded) bias here.
    y = np.zeros((B, C, S), np.float32)
    for c in range(NCORES):
        b = c // NH
        den = results[c]["yden"].reshape(1, S)
        y[b] += results[c]["y"].reshape(C, S) / den \
            + bo_effs[c].reshape(C, 1)
    return y.reshape(B, C, HGT, WID)


def kernel(**inputs):
    nc = _get_nc()
    in_maps, bo_effs = make_in_maps(**inputs)
    res = run_bass_kernel_spmd(nc, in_maps, list(range(NCORES)))
    return assemble_output(res.results, bo_effs)


if __name__ == "__main__":
    nc = build()
    print("built + compiled ok")


# revision 36
# speedup vs baseline: 1.0869x; 1.0869x over previous
"""Cross-attention kernel for Trainium2, sharded over 8 NeuronCores.

Problem (per reference):
  q = wq @ x_q + bq ; k = wk @ x_kv + bk ; v = wv @ x_kv + bv   (1x1 convs)
  per head: attn = softmax(q^T k / sqrt(hd)) ; out = attn @ v^T
  y = wo @ out + bo

Sharding: core c -> (batch b = c // 4, head n = c % 4). Each core runs one
head's full attention and produces the partial output projection
y_part = wo[:, head] @ out_head; the host sums the 4 head partials per batch.

Device-side simplifications (all mathematically exact):
  * bk drops out entirely: a per-query constant shift of the logits cancels
    in softmax.
  * bv folds into the output bias: sum_j softmax_ij = 1, so v-bias
    contributes wo_col @ bv, added to bo on the host.
  * scale 1/8 folds into wq/bq on the host.
  * no max-subtraction: logits are ~N(0,1) (max |logit| < ~6), exp is safe
    in fp32.
  * softmax denominator comes from a ones-column appended to v^T in the AV
    matmul; normalization happens after AV on [64, S] instead of [S, S].

Layouts: logits are computed transposed, S^T[j, i] (k stationary, q moving),
so the exp'd tile feeds the AV matmul directly with j on partitions — no
transposes anywhere. v^T is produced directly by using x_kv chunks as the
stationary operand of the v projection.
"""

import numpy as np
import ml_dtypes

import concourse.bacc as bacc
import concourse.mybir as mybir
import concourse.tile as tile
from concourse.masks import make_identity
from concourse.bass_utils import run_bass_kernel_spmd

F32 = mybir.dt.float32
BF16 = mybir.dt.bfloat16

B, C, HGT, WID = 2, 256, 64, 64
S = HGT * WID  # 4096 pixels
NH, HD = 4, 64
NCORES = 8
P = 128
IC = 1024  # i-chunk width (2 PSUM banks)
NI = S // IC  # 4
NJ = S // P  # 32 j-blocks
SCALE = HD ** -0.5
EXP_W = 1024  # free width of one exp instruction (must divide IC)


def _emit(tc):
    nc = tc.nc
    xq = nc.dram_tensor("xq", [2, P, S], BF16, kind="ExternalInput").ap()
    xkv = nc.dram_tensor("xkv", [2, P, S], BF16, kind="ExternalInput").ap()
    wqT = nc.dram_tensor("wqT", [2, P, HD], BF16, kind="ExternalInput").ap()
    wkvT = nc.dram_tensor("wkvT", [2, P, P], BF16, kind="ExternalInput").ap()
    woT = nc.dram_tensor("woT", [HD, C], BF16, kind="ExternalInput").ap()
    bq = nc.dram_tensor("bq", [HD, 1], F32, kind="ExternalInput").ap()
    y = nc.dram_tensor("y", [2, P, S], F32, kind="ExternalOutput").ap()
    yden = nc.dram_tensor("yden", [1, S], F32, kind="ExternalOutput").ap()

    with (
        tc.tile_pool(name="const", bufs=1) as cpool,
        tc.tile_pool(name="xp", bufs=1) as xpool,
        tc.tile_pool(name="qkv", bufs=1) as qpool,
        tc.tile_pool(name="es", bufs=3) as epool,
        tc.tile_pool(name="epi", bufs=2) as fpool,
        tc.tile_pool(name="ps", bufs=2, space="PSUM") as pp,
    ):
        # ---- weights / constants into SBUF ----
        wq_sb = cpool.tile([P, 2 * HD], BF16)
        wkv_sb = cpool.tile([P, 2 * P], BF16)
        # weights go via the gpsimd SWDGE queue — a third DMA path that
        # doesn't block the two HWDGE rings carrying the 4MB of activations
        for cch in range(2):
            nc.gpsimd.dma_start(wq_sb[:, cch * HD:(cch + 1) * HD], wqT[cch])
            nc.gpsimd.dma_start(wkv_sb[:, cch * P:(cch + 1) * P], wkvT[cch])
        wo_sb = cpool.tile([HD, C], BF16)
        nc.gpsimd.dma_start(wo_sb[:], woT)
        bq_sb = cpool.tile([HD, 1], F32)
        nc.gpsimd.dma_start(bq_sb[:], bq)
        # Zero bias for exp via memset: a float bias would become a DMA'd
        # const tensor whose transfer queues behind the 4MB input DMAs,
        # delaying the first exp (and idling the PE into a HAM down-clock).
        zbias_sb = cpool.tile([P, 1], F32)
        nc.vector.memset(zbias_sb[:], 0.0)
        # PE warmup burst: ~3.5us of dense matmuls on scratch data while the
        # input DMAs are still in flight. The activity monitor promotes the
        # PE to 2.4GHz after ~3.4us of sustained streaming; without this the
        # whole projection phase runs at the cold 1.2GHz clock.
        wrm_sb = cpool.tile([P, 512], BF16)
        nc.vector.memset(wrm_sb[:], 0.0)
        ident_sb = cpool.tile([P, P], BF16)
        make_identity(nc, ident_sb[:])
        for w in range(3):
            wp = pp.tile([P, 512], F32, tag="s", bufs=2, name="wp")
            nc.tensor.matmul(wp[:], wrm_sb[:, 0:P], wrm_sb[:],
                             start=True, stop=True)
        # Warmup exp so the ~2.7us activation-table load happens during the
        # projection phase, not in front of the first real exp.
        warm_sb = cpool.tile([P, 1], BF16)
        nc.scalar.activation(warm_sb[:], zbias_sb[:],
                             mybir.ActivationFunctionType.Exp,
                             bias=zbias_sb[:])

        # ---- activations into SBUF ----
        xq_sb = [xpool.tile([P, S], BF16, tag=f"xq{i}", name=f"xq_sb{i}")
                 for i in range(2)]
        xkv_sb = [xpool.tile([P, S], BF16, tag=f"xkv{i}", name=f"xkv_sb{i}")
                  for i in range(2)]
        # x_kv lands first (k and v^T projections run first); quarter-DMAs
        # let the projections start before the whole tensor arrives.
        QW = S // 2
        # Chunk 0 on the sync HWDGE ring, chunk 1 on the scalar-engine ring:
        # both rings stream in parallel, halving time-to-arrival. Half-sized
        # transfers: arriving a bit later but with fewer per-DMA overheads —
        # the projections then stream without arrival gaps (a >3.4us PE gap
        # would demote the clock).
        for qt in range(2):
            qsl = slice(qt * QW, (qt + 1) * QW)
            nc.sync.dma_start(xkv_sb[0][:, qsl], xkv[0][:, qsl])
            nc.scalar.dma_start(xkv_sb[1][:, qsl], xkv[1][:, qsl])
        for qt in range(2):
            qsl = slice(qt * QW, (qt + 1) * QW)
            nc.sync.dma_start(xq_sb[0][:, qsl], xq[0][:, qsl])
            nc.scalar.dma_start(xq_sb[1][:, qsl], xq[1][:, qsl])

        # q/k are zero-padded to 128 partitions and v^T blocks to 128
        # columns so every LDWEIGHTS is a full [128,128] bf16 load (fast
        # weight-load eligible); the zero rows contribute nothing.
        q_sb = qpool.tile([P, S], BF16)
        k_sb = qpool.tile([P, S], BF16)
        # v^T blocks: [j-part, (block, 128)]; col 64 = ones (denominator),
        # cols 65:128 zero.
        va_sb = qpool.tile([P, NJ * P], BF16)
        nc.vector.memset(va_sb[:], 0.0)
        va_v = va_sb.rearrange("p (j c) -> p j c", c=P)
        nc.vector.memset(va_v[:, :, HD:HD + 1], 1.0)
        # pad memsets run here: the DVE is idle while the input DMAs stream
        nc.vector.memset(q_sb[HD:P, :], 0.0)
        nc.vector.memset(k_sb[HD:P, :], 0.0)

        # ---- fused k+v projection: stationary [wk^T | wv^T] gives k in
        # PSUM rows 0:64 and v in rows 64:128 from the same matmul pass
        # (dense, high PE duty: together with the warmup burst this promotes
        # the PE clock early). v is then moved into the [j-part, block, hd]
        # layout the AV matmul needs by one hardware DMA transpose — a
        # direct x_kv-stationary v^T projection would be 64 tiny matmuls
        # dominated by LDWEIGHTS (~15us of low-duty PE time).
        v_sb = qpool.tile([P, S], BF16)
        for t in range(S // 512):
            sl = slice(t * 512, (t + 1) * 512)
            kvp = pp.tile([P, 512], F32, tag="s", bufs=2)
            nc.tensor.matmul(kvp[:], wkv_sb[:, 0:P], xkv_sb[0][:, sl],
                             start=True, stop=False)
            nc.tensor.matmul(kvp[:], wkv_sb[:, P:2 * P], xkv_sb[1][:, sl],
                             start=False, stop=True)
            nc.vector.tensor_copy(k_sb[0:HD, sl], kvp[0:HD, :])
            # v copy runs on the (otherwise idle) scalar engine
            nc.scalar.activation(v_sb[HD:P, sl], kvp[HD:P, :],
                                 mybir.ActivationFunctionType.Copy)
            # transpose v into the AV layout with PE transpose ops as each
            # 512-col span lands (a DMA transpose would queue behind the
            # input DMAs and finish after the first AV matmul needs it)
            for j in range(t * 4, (t + 1) * 4):
                jb = slice(j * P, (j + 1) * P)
                vt = pp.tile([P, HD], BF16, tag="s", bufs=2, name="vt")
                nc.tensor.transpose(vt[:], v_sb[HD:P, jb],
                                    ident_sb[HD:P, HD:P])
                nc.vector.tensor_copy(va_v[:, j, 0:HD], vt[:])

        # ---- q projection (scale and bias folded in on the host) ----
        for t in range(S // 512):
            sl = slice(t * 512, (t + 1) * 512)
            qp = pp.tile([HD, 512], F32, tag="s", bufs=2)
            nc.tensor.matmul(qp[:], wq_sb[:, 0:HD], xq_sb[0][:, sl],
                             start=True, stop=False)
            nc.tensor.matmul(qp[:], wq_sb[:, HD:2 * HD], xq_sb[1][:, sl],
                             start=False, stop=True)
            nc.vector.tensor_scalar_add(q_sb[0:HD, sl], qp[:], bq_sb[:])



        # ---- attention, i-chunk at a time ----
        # Softmax normalization is deferred to the host: division by the
        # denominator commutes with the output projection, so the device
        # ships y_un = wo_col @ (exp(S^T)^T V)^T plus the per-pixel
        # denominators, and the host computes y_un / den + bias. This keeps
        # any long dependency chain (reciprocal etc.) out of the in-order PE
        # stream — a multi-us PE stall makes the HW activity monitor halve
        # the PE clock for the rest of the kernel.
        pend = [None] * NI  # per chunk: unnormalized out^T awaiting out-proj

        def epilogue_part2(i):
            # out-projection of the (unnormalized) attention output
            outt = pend[i]
            for oh in range(2):
                for h in range(IC // 512):
                    yp = pp.tile([P, 512], F32, tag="av", bufs=2, name="yp")
                    nc.tensor.matmul(yp[:], wo_sb[:, oh * P:(oh + 1) * P],
                                     outt[:, h * 512:(h + 1) * 512],
                                     start=True, stop=True)
                    ys = fpool.tile([P, 512], F32, name="ys")
                    nc.vector.tensor_copy(ys[:], yp[:])
                    eng = nc.sync if oh == 0 else nc.scalar
                    eng.dma_start(
                        y[oh][:, i * IC + h * 512:i * IC + (h + 1) * 512],
                        ys[:])

        for i in range(NI):
            av = pp.tile([P, IC], F32, tag="av", bufs=2)
            for j in range(NJ):
                if i > 0 and j == 8:
                    epilogue_part2(i - 1)
                jb = slice(j * P, (j + 1) * P)
                st = pp.tile([P, IC], F32, tag="s", bufs=2)
                for h in range(IC // 512):
                    isl = slice(i * IC + h * 512, i * IC + (h + 1) * 512)
                    nc.tensor.matmul(st[:, h * 512:(h + 1) * 512],
                                     k_sb[:, jb], q_sb[:, isl],
                                     start=True, stop=True)
                et = epool.tile([P, IC], BF16)
                for h in range(IC // EXP_W):
                    esl = slice(h * EXP_W, (h + 1) * EXP_W)
                    nc.scalar.activation(et[:, esl], st[:, esl],
                                         mybir.ActivationFunctionType.Exp,
                                         bias=zbias_sb[:])
                vbase = j * P
                for h in range(IC // 512):
                    nc.tensor.matmul(av[:, h * 512:(h + 1) * 512],
                                     va_sb[:, vbase:vbase + P],
                                     et[:, h * 512:(h + 1) * 512],
                                     start=(j == 0), stop=(j == NJ - 1))

            # epilogue part 1: DVE only — drain PSUM and free the av slot
            outt = fpool.tile([HD, IC], BF16)
            nc.vector.tensor_copy(outt[:], av[0:HD, :])
            den = fpool.tile([HD + 1, IC], F32, name="den")
            nc.vector.tensor_copy(den[HD:HD + 1, :], av[HD:HD + 1, :])
            nc.gpsimd.dma_start(yden[:, i * IC:(i + 1) * IC],
                                den[HD:HD + 1, :])
            pend[i] = outt

        epilogue_part2(NI - 1)


def build():
    nc = bacc.Bacc("TRN2", target_bir_lowering=False, debug=False,
                   enable_asserts=False)
    with tile.TileContext(nc) as tc:
        _emit(tc)
    nc.compile()
    return nc


_NC_CACHE = []


def _get_nc():
    if not _NC_CACHE:
        _NC_CACHE.append(build())
    return _NC_CACHE[0]


def make_in_maps(x_q, x_kv, wq, bq, wk, bk, wv, bv, wo, bo):
    bf = ml_dtypes.bfloat16
    in_maps = []
    bo_effs = []
    for c in range(NCORES):
        b, n = divmod(c, NH)
        hs = slice(n * HD, (n + 1) * HD)
        wq_h = wq[hs].astype(np.float64) * SCALE
        bo_eff = wo[:, hs].astype(np.float64) @ bv[hs].astype(np.float64)
        if n == 0:
            bo_eff = bo_eff + bo.astype(np.float64)
        bo_effs.append(bo_eff.astype(np.float32))
        in_maps.append({
            "xq": np.ascontiguousarray(
                x_q[b].reshape(C, S).reshape(2, P, S)).astype(bf),
            "xkv": np.ascontiguousarray(
                x_kv[b].reshape(C, S).reshape(2, P, S)).astype(bf),
            "wqT": np.ascontiguousarray(wq_h.T.reshape(2, P, HD)).astype(bf),
            "wkvT": np.ascontiguousarray(
                np.concatenate([wk[hs].T, wv[hs].T], axis=1)
                .reshape(2, P, P)).astype(bf),
            "woT": np.ascontiguousarray(wo[:, hs].T).astype(bf),
            "bq": (bq[hs].astype(np.float64) * SCALE
                   ).astype(np.float32).reshape(HD, 1),
        })
    return in_maps, bo_effs


def assemble_output(results, bo_effs):
    # y_core is the unnormalized head partial; divide by the softmax
    # denominator and add the (host-folded) bias here.
    y = np.zeros((B, C, S), np.float32)
    for c in range(NCORES):
        b = c // NH
        den = results[c]["yden"].reshape(1, S)
        y[b] += results[c]["y"].reshape(C, S) / den \
            + bo_effs[c].reshape(C, 1)
    return y.reshape(B, C, HGT, WID)


def kernel(**inputs):
    nc = _get_nc()
    in_maps, bo_effs = make_in_maps(**inputs)
    res = run_bass_kernel_spmd(nc, in_maps, list(range(NCORES)))
    return assemble_output(res.results, bo_effs)


if __name__ == "__main__":
    nc = build()
    print("built + compiled ok")


# revision 37
# speedup vs baseline: 1.1906x; 1.0955x over previous
"""Cross-attention kernel for Trainium2, sharded over 8 NeuronCores.

Problem (per reference):
  q = wq @ x_q + bq ; k = wk @ x_kv + bk ; v = wv @ x_kv + bv   (1x1 convs)
  per head: attn = softmax(q^T k / sqrt(hd)) ; out = attn @ v^T
  y = wo @ out + bo

Sharding: core c -> (batch b = c // 4, head n = c % 4). Each core runs one
head's full attention and produces the partial output projection
y_part = wo[:, head] @ out_head; the host sums the 4 head partials per batch.

Device-side simplifications (all mathematically exact):
  * bk drops out entirely: a per-query constant shift of the logits cancels
    in softmax.
  * bv folds into the output bias: sum_j softmax_ij = 1, so v-bias
    contributes wo_col @ bv, added to bo on the host.
  * scale 1/8 folds into wq/bq on the host.
  * no max-subtraction: logits are ~N(0,1) (max |logit| < ~6), exp is safe
    in fp32.
  * softmax denominator comes from a ones-column appended to v^T in the AV
    matmul; normalization happens after AV on [64, S] instead of [S, S].

Layouts: logits are computed transposed, S^T[j, i] (k stationary, q moving),
so the exp'd tile feeds the AV matmul directly with j on partitions — no
transposes anywhere. v^T is produced directly by using x_kv chunks as the
stationary operand of the v projection.
"""

import numpy as np
import ml_dtypes

import concourse.bacc as bacc
import concourse.mybir as mybir
import concourse.tile as tile
from concourse.bass_utils import run_bass_kernel_spmd

F32 = mybir.dt.float32
BF16 = mybir.dt.bfloat16

B, C, HGT, WID = 2, 256, 64, 64
S = HGT * WID  # 4096 pixels
NH, HD = 4, 64
NCORES = 8
P = 128
IC = 1024  # i-chunk width (2 PSUM banks)
NI = S // IC  # 4
NJ = S // P  # 32 j-blocks
SCALE = HD ** -0.5
EXP_W = 1024  # free width of one exp instruction (must divide IC)


def _emit(tc):
    nc = tc.nc
    xq = nc.dram_tensor("xq", [2, P, S], BF16, kind="ExternalInput").ap()
    xkv = nc.dram_tensor("xkv", [2, P, S], BF16, kind="ExternalInput").ap()
    wqT = nc.dram_tensor("wqT", [2, P, HD], BF16, kind="ExternalInput").ap()
    wkvT = nc.dram_tensor("wkvT", [2, P, P], BF16, kind="ExternalInput").ap()
    woT = nc.dram_tensor("woT", [HD, C], BF16, kind="ExternalInput").ap()
    bq = nc.dram_tensor("bq", [HD, 1], F32, kind="ExternalInput").ap()
    y = nc.dram_tensor("y", [2, P, S], F32, kind="ExternalOutput").ap()
    yden = nc.dram_tensor("yden", [1, S], F32, kind="ExternalOutput").ap()

    with (
        tc.tile_pool(name="const", bufs=1) as cpool,
        tc.tile_pool(name="xp", bufs=1) as xpool,
        tc.tile_pool(name="qkv", bufs=1) as qpool,
        tc.tile_pool(name="es", bufs=10) as epool,
        tc.tile_pool(name="epi", bufs=2) as fpool,
        tc.tile_pool(name="ps", bufs=2, space="PSUM") as pp,
    ):
        # ---- weights / constants into SBUF ----
        wq_sb = cpool.tile([P, 2 * HD], BF16)
        wkv_sb = cpool.tile([P, 2 * P], BF16)
        # weights go via the gpsimd SWDGE queue — a third DMA path that
        # doesn't block the two HWDGE rings carrying the 4MB of activations
        for cch in range(2):
            nc.gpsimd.dma_start(wq_sb[:, cch * HD:(cch + 1) * HD], wqT[cch])
            nc.gpsimd.dma_start(wkv_sb[:, cch * P:(cch + 1) * P], wkvT[cch])
        wo_sb = cpool.tile([HD, C], BF16)
        nc.gpsimd.dma_start(wo_sb[:], woT)
        bq_sb = cpool.tile([HD, 1], F32)
        nc.gpsimd.dma_start(bq_sb[:], bq)
        # Zero bias for exp via memset: a float bias would become a DMA'd
        # const tensor whose transfer queues behind the 4MB input DMAs,
        # delaying the first exp (and idling the PE into a HAM down-clock).
        zbias_sb = cpool.tile([P, 1], F32)
        nc.vector.memset(zbias_sb[:], 0.0)
        # PE warmup burst: ~3.5us of dense matmuls on scratch data while the
        # input DMAs are still in flight. The activity monitor promotes the
        # PE to 2.4GHz after ~3.4us of sustained streaming; without this the
        # whole projection phase runs at the cold 1.2GHz clock.
        wrm_sb = cpool.tile([P, 512], BF16)
        nc.vector.memset(wrm_sb[:], 0.0)
        for w in range(3):
            wp = pp.tile([P, 512], F32, tag="s", bufs=2, name="wp")
            nc.tensor.matmul(wp[:], wrm_sb[:, 0:P], wrm_sb[:],
                             start=True, stop=True)
        # Warmup exp so the ~2.7us activation-table load happens during the
        # projection phase, not in front of the first real exp.
        warm_sb = cpool.tile([P, 1], BF16)
        nc.scalar.activation(warm_sb[:], zbias_sb[:],
                             mybir.ActivationFunctionType.Exp,
                             bias=zbias_sb[:])

        # ---- activations into SBUF ----
        xq_sb = [xpool.tile([P, S], BF16, tag=f"xq{i}", name=f"xq_sb{i}")
                 for i in range(2)]
        xkv_sb = [xpool.tile([P, S], BF16, tag=f"xkv{i}", name=f"xkv_sb{i}")
                  for i in range(2)]
        # x_kv lands first (k and v^T projections run first); quarter-DMAs
        # let the projections start before the whole tensor arrives.
        QW = S // 2
        # Chunk 0 on the sync HWDGE ring, chunk 1 on the scalar-engine ring:
        # both rings stream in parallel, halving time-to-arrival. Half-sized
        # transfers: arriving a bit later but with fewer per-DMA overheads —
        # the projections then stream without arrival gaps (a >3.4us PE gap
        # would demote the clock).
        for qt in range(2):
            qsl = slice(qt * QW, (qt + 1) * QW)
            nc.sync.dma_start(xkv_sb[0][:, qsl], xkv[0][:, qsl])
            nc.scalar.dma_start(xkv_sb[1][:, qsl], xkv[1][:, qsl])
        for qt in range(2):
            qsl = slice(qt * QW, (qt + 1) * QW)
            nc.sync.dma_start(xq_sb[0][:, qsl], xq[0][:, qsl])
            nc.scalar.dma_start(xq_sb[1][:, qsl], xq[1][:, qsl])

        # q/k are zero-padded to 128 partitions and v^T blocks to 128
        # columns so every LDWEIGHTS is a full [128,128] bf16 load (fast
        # weight-load eligible); the zero rows contribute nothing.
        q_sb = qpool.tile([P, S], BF16)
        k_sb = qpool.tile([P, S], BF16)
        # v^T blocks: [j-part, (block, 128)]; col 64 = ones (denominator),
        # cols 65:128 zero.
        va_sb = qpool.tile([P, NJ * P], BF16)
        nc.vector.memset(va_sb[:], 0.0)
        va_v = va_sb.rearrange("p (j c) -> p j c", c=P)
        nc.vector.memset(va_v[:, :, HD:HD + 1], 1.0)
        # pad memsets run here: the DVE is idle while the input DMAs stream
        nc.vector.memset(q_sb[HD:P, :], 0.0)
        nc.vector.memset(k_sb[HD:P, :], 0.0)

        # ---- fused k+v projection: stationary [wk^T | wv^T] gives k in
        # PSUM rows 0:64 and v in rows 64:128 from the same matmul pass
        # (dense, high PE duty: together with the warmup burst this promotes
        # the PE clock early). v is then moved into the [j-part, block, hd]
        # layout the AV matmul needs by one hardware DMA transpose — a
        # direct x_kv-stationary v^T projection would be 64 tiny matmuls
        # dominated by LDWEIGHTS (~15us of low-duty PE time).
        v_sb = qpool.tile([P, S], BF16)
        for t in range(S // 512):
            sl = slice(t * 512, (t + 1) * 512)
            kvp = pp.tile([P, 512], F32, tag="s", bufs=2)
            nc.tensor.matmul(kvp[:], wkv_sb[:, 0:P], xkv_sb[0][:, sl],
                             start=True, stop=False)
            nc.tensor.matmul(kvp[:], wkv_sb[:, P:2 * P], xkv_sb[1][:, sl],
                             start=False, stop=True)
            nc.vector.tensor_copy(k_sb[0:HD, sl], kvp[0:HD, :])
            # v copy runs on the (otherwise idle) scalar engine
            nc.scalar.activation(v_sb[HD:P, sl], kvp[HD:P, :],
                                 mybir.ActivationFunctionType.Copy)
        nc.sync.dma_start_transpose(out=va_v[:, :, 0:HD], in_=v_sb[HD:P, :])


        # ---- q projection (scale and bias folded in on the host) ----
        for t in range(S // 512):
            sl = slice(t * 512, (t + 1) * 512)
            qp = pp.tile([HD, 512], F32, tag="s", bufs=2)
            nc.tensor.matmul(qp[:], wq_sb[:, 0:HD], xq_sb[0][:, sl],
                             start=True, stop=False)
            nc.tensor.matmul(qp[:], wq_sb[:, HD:2 * HD], xq_sb[1][:, sl],
                             start=False, stop=True)
            nc.vector.tensor_scalar_add(q_sb[0:HD, sl], qp[:], bq_sb[:])



        # ---- attention, i-chunk at a time ----
        # Softmax normalization is deferred to the host: division by the
        # denominator commutes with the output projection, so the device
        # ships y_un = wo_col @ (exp(S^T)^T V)^T plus the per-pixel
        # denominators, and the host computes y_un / den + bias. This keeps
        # any long dependency chain (reciprocal etc.) out of the in-order PE
        # stream — a multi-us PE stall makes the HW activity monitor halve
        # the PE clock for the rest of the kernel.
        pend = [None] * NI  # per chunk: unnormalized out^T awaiting out-proj

        def epilogue_part2(i):
            # out-projection of the (unnormalized) attention output
            outt = pend[i]
            for oh in range(2):
                for h in range(IC // 512):
                    yp = pp.tile([P, 512], F32, tag="av", bufs=2, name="yp")
                    nc.tensor.matmul(yp[:], wo_sb[:, oh * P:(oh + 1) * P],
                                     outt[:, h * 512:(h + 1) * 512],
                                     start=True, stop=True)
                    ys = fpool.tile([P, 512], F32, name="ys")
                    nc.vector.tensor_copy(ys[:], yp[:])
                    eng = nc.sync if oh == 0 else nc.scalar
                    eng.dma_start(
                        y[oh][:, i * IC + h * 512:i * IC + (h + 1) * 512],
                        ys[:])

        for i in range(NI):
            av = pp.tile([P, IC], F32, tag="av", bufs=2)
            for j in range(NJ):
                if i > 0 and j == 8:
                    epilogue_part2(i - 1)
                jb = slice(j * P, (j + 1) * P)
                st = pp.tile([P, IC], F32, tag="s", bufs=2)
                for h in range(IC // 512):
                    isl = slice(i * IC + h * 512, i * IC + (h + 1) * 512)
                    nc.tensor.matmul(st[:, h * 512:(h + 1) * 512],
                                     k_sb[:, jb], q_sb[:, isl],
                                     start=True, stop=True)
                et = epool.tile([P, IC], BF16)
                for h in range(IC // EXP_W):
                    esl = slice(h * EXP_W, (h + 1) * EXP_W)
                    nc.scalar.activation(et[:, esl], st[:, esl],
                                         mybir.ActivationFunctionType.Exp,
                                         bias=zbias_sb[:])
                vbase = j * P
                for h in range(IC // 512):
                    nc.tensor.matmul(av[:, h * 512:(h + 1) * 512],
                                     va_sb[:, vbase:vbase + P],
                                     et[:, h * 512:(h + 1) * 512],
                                     start=(j == 0), stop=(j == NJ - 1))

            # epilogue part 1: DVE only — drain PSUM and free the av slot
            outt = fpool.tile([HD, IC], BF16)
            nc.vector.tensor_copy(outt[:], av[0:HD, :])
            den = fpool.tile([HD + 1, IC], F32, name="den")
            nc.vector.tensor_copy(den[HD:HD + 1, :], av[HD:HD + 1, :])
            nc.gpsimd.dma_start(yden[:, i * IC:(i + 1) * IC],
                                den[HD:HD + 1, :])
            pend[i] = outt

        epilogue_part2(NI - 1)


def build():
    nc = bacc.Bacc("TRN2", target_bir_lowering=False, debug=False,
                   enable_asserts=False)
    with tile.TileContext(nc) as tc:
        _emit(tc)
    nc.compile()
    return nc


_NC_CACHE = []


def _get_nc():
    if not _NC_CACHE:
        _NC_CACHE.append(build())
    return _NC_CACHE[0]


def make_in_maps(x_q, x_kv, wq, bq, wk, bk, wv, bv, wo, bo):
    bf = ml_dtypes.bfloat16
    in_maps = []
    bo_effs = []
    for c in range(NCORES):
        b, n = divmod(c, NH)
        hs = slice(n * HD, (n + 1) * HD)
        wq_h = wq[hs].astype(np.float64) * SCALE
        bo_eff = wo[:, hs].astype(np.float64) @ bv[hs].astype(np.float64)
        if n == 0:
            bo_eff = bo_eff + bo.astype(np.float64)
        bo_effs.append(bo_eff.astype(np.float32))
        in_maps.append({
            "xq": np.ascontiguousarray(
                x_q[b].reshape(C, S).reshape(2, P, S)).astype(bf),
            "xkv": np.ascontiguousarray(
                x_kv[b].reshape(C, S).reshape(2, P, S)).astype(bf),
            "wqT": np.ascontiguousarray(wq_h.T.reshape(2, P, HD)).astype(bf),
            "wkvT": np.ascontiguousarray(
                np.concatenate([wk[hs].T, wv[hs].T], axis=1)
                .reshape(2, P, P)).astype(bf),
            "woT": np.ascontiguousarray(wo[:, hs].T).astype(bf),
            "bq": (bq[hs].astype(np.float64) * SCALE
                   ).astype(np.float32).reshape(HD, 1),
        })
    return in_maps, bo_effs


def assemble_output(results, bo_effs):
    # y_core is the unnormalized head partial; divide by the softmax
    # denominator and add the (host-folded) bias here.
    y = np.zeros((B, C, S), np.float32)
    for c in range(NCORES):
        b = c // NH
        den = results[c]["yden"].reshape(1, S)
        y[b] += results[c]["y"].reshape(C, S) / den \
            + bo_effs[c].reshape(C, 1)
    return y.reshape(B, C, HGT, WID)


def kernel(**inputs):
    nc = _get_nc()
    in_maps, bo_effs = make_in_maps(**inputs)
    res = run_bass_kernel_spmd(nc, in_maps, list(range(NCORES)))
    return assemble_output(res.results, bo_effs)


if __name__ == "__main__":
    nc = build()
    print("built + compiled ok")
